# revision 5
# baseline (speedup 1.0000x reference)
"""Trainium2 Bass kernel for nn_Contrastive_D (contrastive + mapper/entropy loss).

Self-contained: hardcodes shapes from the problem spec.
  b, z: [8192, 128] f32; y: [8192] int; W1 [16,8,256]; b1 [16,256];
  W2 [16,256,256]; b2 [16,256]; perm_idx [128]; flip masks [8192,128] bool.
Returns the scalar loss (f32).

v2 changes vs v1:
  - mapLoss target-select (was DVE iota-compare js) is now a trace via
    one-hot matmul: S = sum_{m,t} Tsel_m.T @ dec_m accumulated in PSUM,
    diagonal extracted once at the end.  Host builds Tsel (one-hot of the
    per-(row, subnet) target byte) - index bookkeeping like the one-hot Y.
  - contrastive (C) and mapper-L2 (B1) are emitted interleaved so the
    DVE-heavy C work and PE-heavy B1 work overlap; all PSUM pools coexist
    in one scope (psC 4 banks + dec 2 + trace-S 1 + seg 1 = 8 banks).
  - B1 evacuates dec via DVE only (ACT queue is busy with contrastive
    exps); sev sums come from the bf16 dec copy via ACT exp+accum_out.
  - host ships zflip.T directly; target bytes computed host-side.
  - pb target-logit dots via fused tensor_tensor_reduce.
"""

import os
import numpy as np
import ml_dtypes

import concourse.bass as bass
import concourse.bacc as bacc
import concourse.mybir as mybir
import concourse.tile as tile
from concourse.bass_utils import run_bass_kernel_spmd

F32 = mybir.dt.float32
BF16 = mybir.dt.bfloat16
AF = mybir.ActivationFunctionType
ALU = mybir.AluOpType
AX = mybir.AxisListType

NCORES = 8
N = 8192
BITS = 128
M = 16
C = 100
TEMP = 0.07
LAM = 0.5
ITEMP = 1.0 / TEMP
R = N // NCORES          # rows per core
NT = R // 128            # 128-row tiles per core
NEG_BIG = 65536.0        # diag mask subtrahend

_CACHE = {}

bf16 = ml_dtypes.bfloat16

# CoreSim doesn't implement the Silu ACT function; the sim checker flips this
# to build an equivalent sigmoid+multiply variant (hardware uses native Silu).
SILU_VIA_SIGMOID = False
# hw-debug toggles
TT_DMA_ON_ACT = True     # issue Tsel loads from the ACT HWDGE queue
PER_Q_AR = True          # 4 pipelined per-q AllReduces vs one big one
USE_TTR = False          # fused ttr ucode faults this hw (NRT_EXEC_UNIT_UNRECOVERABLE)


def _build_program(sim_mode=False):
    # sim_mode: replace the cross-core collective with a local DMA copy so the
    # single-core TimelineSim can model the program (analysis tooling only —
    # the real kernel path always uses sim_mode=False).
    key = "nc_sim" if sim_mode else "nc"
    if key in _CACHE:
        return _CACHE[key]
    nc = bacc.Bacc(
        "TRN2", target_bir_lowering=False, debug=False, num_devices=NCORES
    )

    def inp(name, shape, dtype):
        return nc.dram_tensor(name, shape, dtype, kind="ExternalInput")

    bT = inp("bTrot", [128, N], BF16)          # b.T, columns rotated per core
    btT = inp("btT", [128, R], BF16)           # b[t_idx].T shard
    zfT = inp("zfT", [128, R], BF16)           # zflip.T shard (host-flipped)
    Yb = inp("Yb", [R, C], BF16)               # one-hot(y) shard
    Tsel = inp("Tsel", [128, 32 * 1024], BF16)  # one-hot(target), pre-tiled layout
    W1bd = inp("W1bd", [128, 4096], BF16)      # block-diag layer-1 weights
    W2s = inp("W2s", [128, 32 * 256], BF16)    # layer-2 weights, [k, (2m+hc)*256+o]
    b1c = inp("b1c", [128, 32], F32)           # layer-1 bias, feature-major blocks
    b2r = inp("b2r", [1, 4096], BF16)          # layer-2 bias row
    bgI = inp("bigI", [128, 128], BF16)        # NEG_BIG * identity
    dgm = inp("diagm", [128, 512], BF16)       # S-diag masks, both v-halves
    ivc = inp("invc", [128, 1], F32)           # 1/clip(counts,1), padded to 128
    cps = inp("cpos", [128, 1], F32)           # counts>0 mask, padded
    onf = inp("ones_f", [128, 1], F32)
    onr = inp("ones_r", [1, 128], BF16)

    outv = nc.dram_tensor("outv", [1, 8], F32, kind="ExternalOutput")

    with tile.TileContext(nc) as tc:
        with (
            tc.tile_pool(name="cst", bufs=1) as cst,
            tc.tile_pool(name="dram", bufs=1, space="DRAM") as dram,
        ):
            # ---------------- constant / persistent SBUF ----------------
            # phase-A inputs load first so silu work starts ASAP
            zf_sb = cst.tile([128, R], BF16)
            nc.sync.dma_start(zf_sb[:], zfT[:])
            W1_sb = cst.tile([128, 4096], BF16)
            nc.sync.dma_start(W1_sb[:], W1bd[:])
            b1_sb = cst.tile([128, 32], F32)
            nc.sync.dma_start(b1_sb[:], b1c[:])
            bT_sb = cst.tile([128, N], BF16)
            for ch in range(N // 2048):
                nc.sync.dma_start(
                    bT_sb[:, ch * 2048:(ch + 1) * 2048],
                    bT[:, ch * 2048:(ch + 1) * 2048],
                )
            bt_sb = cst.tile([128, R], BF16)
            nc.sync.dma_start(bt_sb[:], btT[:])
            Y_sb = cst.tile([128, NT * C], BF16)
            for t in range(NT):
                nc.sync.dma_start(
                    Y_sb[:, t * C:(t + 1) * C], Yb[t * 128:(t + 1) * 128, :]
                )
            b2_sb = cst.tile([1, 4096], BF16)
            nc.sync.dma_start(b2_sb[:], b2r[:])
            bI_sb = cst.tile([128, 128], BF16)
            nc.sync.dma_start(bI_sb[:], bgI[:])
            dg_sb = cst.tile([128, 512], BF16)
            nc.sync.dma_start(dg_sb[:], dgm[:])
            ic_sb = cst.tile([128, 1], F32)
            nc.sync.dma_start(ic_sb[:], ivc[:])
            cp_sb = cst.tile([128, 1], F32)
            nc.sync.dma_start(cp_sb[:], cps[:])
            of_sb = cst.tile([128, 1], F32)
            nc.sync.dma_start(of_sb[:], onf[:])
            or_sb = cst.tile([1, 128], BF16)
            nc.sync.dma_start(or_sb[:], onr[:])

            hT_sb = cst.tile([128, 32 * R], BF16)      # silu activations, feat-major
            dcb_sb = cst.tile([128, 32 * 1024], BF16)  # bf16 dec, col (q*NT+t)*1024
            pb_sb = cst.tile([128, NT], F32)           # target-logit dot partials
            mls_sb = cst.tile([128, NT], F32)          # mapLoss per-row partials
            Mcols = cst.tile([128, NT], F32)           # per-tile row maxes
            Scols = cst.tile([128, NT], F32)           # per-tile row sum-exp
            sev = [
                cst.tile([128, 16], BF16, name=f"sev{t}", tag=f"sev{t}")
                for t in range(NT)
            ]
            SmE = cst.tile([128, 16], F32)             # class-entropy sums
            T1E = cst.tile([128, 16], F32)
            Sdm = cst.tile([1, 16], F32)               # dmean-entropy sums
            T1d = cst.tile([1, 16], F32)
            S_sb = cst.tile([128, 512], F32)           # trace-S evacuated
            je_sb = cst.tile([128, 1024], BF16)        # write-only exp scratch
            out_sb = cst.tile([1, 8], F32)
            nc.vector.memset(out_sb[:], 0.0)

            # per-q segments so the AllReduce pipelines with B1's tail
            if PER_Q_AR:
                seg_part = [dram.tile([C, 1024], F32, name=f"segp{q}") for q in range(4)]
                seg_red = [
                    dram.tile([C, 1024], F32, addr_space="Shared", name=f"segr{q}")
                    for q in range(4)
                ]
            else:
                seg_part_f = dram.tile([C, 4096], F32, name="segp")
                seg_red_f = dram.tile([C, 4096], F32, addr_space="Shared", name="segr")
                seg_part = [seg_part_f[:, q * 1024:(q + 1) * 1024] for q in range(4)]
                seg_red = [seg_red_f[:, q * 1024:(q + 1) * 1024] for q in range(4)]

            # ---------------- phase 0: pb target-logit dots ----------------
            with tc.tile_pool(name="w0", bufs=2) as w0:
                for t in range(NT):
                    jp = w0.tile([128, 128], BF16, tag="jp")
                    if USE_TTR:
                        nc.vector.tensor_tensor_reduce(
                            jp[:],
                            bT_sb[:, t * 128:(t + 1) * 128],
                            bt_sb[:, t * 128:(t + 1) * 128],
                            1.0,
                            0.0,
                            ALU.mult,
                            ALU.add,
                            accum_out=pb_sb[:, t:t + 1],
                        )
                    else:
                        nc.vector.tensor_tensor(
                            jp[:],
                            bT_sb[:, t * 128:(t + 1) * 128],
                            bt_sb[:, t * 128:(t + 1) * 128],
                            ALU.mult,
                        )
                        nc.vector.tensor_reduce(
                            pb_sb[:, t:t + 1], jp[:], AX.X, ALU.add
                        )

            # ---------------- phase A: mapper layer 1 ----------------
            with tc.tile_pool(name="psA", bufs=2, space="PSUM") as psA:
                sig = cst.tile([128, R], BF16, name="sig") if SILU_VIA_SIGMOID else None
                for ob in range(32):
                    hp = psA.tile([128, R], F32, tag="hp")
                    for k in range(R // 512):
                        nc.tensor.matmul(
                            hp[:, k * 512:(k + 1) * 512],
                            lhsT=W1_sb[:, ob * 128:(ob + 1) * 128],
                            rhs=zf_sb[:, k * 512:(k + 1) * 512],
                            start=True,
                            stop=True,
                        )
                    if SILU_VIA_SIGMOID:
                        nc.scalar.activation(
                            sig[:], hp[:], AF.Sigmoid,
                            bias=b1_sb[:, ob:ob + 1], scale=1.0,
                        )
                        nc.vector.scalar_tensor_tensor(
                            hT_sb[:, ob * R:(ob + 1) * R],
                            hp[:], b1_sb[:, ob:ob + 1], sig[:],
                            ALU.add, ALU.mult,
                        )
                    else:
                        nc.scalar.activation(
                            hT_sb[:, ob * R:(ob + 1) * R],
                            hp[:],
                            AF.Silu,
                            bias=b1_sb[:, ob:ob + 1],
                            scale=1.0,
                        )

            # -------- phases C + B1 interleaved: contrastive | L2 + trace + seg --------
            def emit_c_tile(t, wC, psC):
                gmx = wC.tile([128, 8], F32, tag="gmx")
                sg8 = wC.tile([128, 8], F32, tag="sg8")
                ngb = wC.tile([128, 8], F32, tag="ngb")
                for g in range(8):
                    lg = psC.tile([128, 1024], F32, tag="lg")
                    for k in range(2):
                        nc.tensor.matmul(
                            lg[:, k * 512:(k + 1) * 512],
                            lhsT=bT_sb[:, t * 128:(t + 1) * 128],
                            rhs=bT_sb[:, g * 1024 + k * 512:g * 1024 + (k + 1) * 512],
                            start=True,
                            stop=True,
                        )
                    if g == 0:
                        nc.vector.tensor_tensor(
                            lg[:, t * 128:(t + 1) * 128],
                            lg[:, t * 128:(t + 1) * 128],
                            bI_sb[:],
                            ALU.subtract,
                        )
                    nc.vector.tensor_reduce(
                        gmx[:, g:g + 1], lg[:], AX.X, ALU.max
                    )
                    nc.vector.tensor_scalar_mul(
                        ngb[:, g:g + 1], gmx[:, g:g + 1], -ITEMP
                    )
                    nc.scalar.activation(
                        je_sb[:],
                        lg[:],
                        AF.Exp,
                        bias=ngb[:, g:g + 1],
                        scale=ITEMP,
                        accum_out=sg8[:, g:g + 1],
                    )
                nc.vector.tensor_reduce(Mcols[:, t:t + 1], gmx[:], AX.X, ALU.max)
                nb1 = wC.tile([128, 1], F32, tag="nb1")
                nc.vector.tensor_scalar_mul(nb1[:], Mcols[:, t:t + 1], -ITEMP)
                wg = wC.tile([128, 8], F32, tag="wg")
                nc.scalar.activation(
                    wg[:], gmx[:], AF.Exp, bias=nb1[:], scale=ITEMP
                )
                j4 = wC.tile([128, 8], F32, tag="j4")
                nc.vector.tensor_tensor(j4[:], sg8[:], wg[:], ALU.mult)
                nc.vector.tensor_reduce(
                    Scols[:, t:t + 1], j4[:], AX.X, ALU.add
                )

            with (
                tc.tile_pool(name="wC", bufs=2) as wC,
                tc.tile_pool(name="wT", bufs=2) as wT,
                tc.tile_pool(name="wW2", bufs=2) as wW2,
                tc.tile_pool(name="wSg", bufs=2) as wSg,
                tc.tile_pool(name="psC", bufs=2, space="PSUM") as psC,
                tc.tile_pool(name="psB", bufs=2, space="PSUM") as psB,
                tc.tile_pool(name="psS1", bufs=1, space="PSUM") as psS1,
                tc.tile_pool(name="psSg", bufs=1, space="PSUM") as psSg,
            ):
                S_ps = psS1.tile([128, 512], F32, tag="Sps")
                n_acc = 0
                # one-unit-delayed emission queues: decouple the in-order
                # engine queues (PE trace-mms wait on DVE dcb copies; DVE
                # sev-reduces wait on ACT esb exps)
                pend_trace = []
                pend_sev = []

                def flush_pending():
                    nonlocal n_acc
                    for args in pend_trace:
                        n_acc += 1
                        nc.tensor.matmul(
                            args[0], lhsT=args[1], rhs=args[2],
                            start=(n_acc <= 2), stop=(n_acc > 254),
                            skip_group_check=True,
                        )
                    pend_trace.clear()
                    for sv, es in pend_sev:
                        with nc.allow_low_precision("sev sums tolerate bf16"):
                            nc.vector.tensor_reduce(
                                sv, es.rearrange("p (m o) -> p m o", o=256),
                                AX.X, ALU.add,
                            )
                    pend_sev.clear()

                for q in range(4):
                    W2q = wW2.tile([128, 2048], BF16, tag="W2q", bufs=1)
                    for ch in range(2):
                        nc.sync.dma_start(
                            W2q[:, ch * 1024:(ch + 1) * 1024],
                            W2s[:, q * 2048 + ch * 1024:q * 2048 + (ch + 1) * 1024],
                        )
                    Tt = None
                    for t in range(NT):
                        i = q * NT + t
                        if i % 4 == 0:
                            emit_c_tile(i // 4, wC, psC)
                        flush_pending()
                        if t % 2 == 0:
                            Tt = wT.tile([128, 2048], BF16, tag="Tt")
                            (nc.scalar if TT_DMA_ON_ACT else nc.sync).dma_start(
                                Tt[:],
                                Tsel[:, (q * NT + t) * 1024:(q * NT + t + 2) * 1024],
                            )
                        tb = (t % 2) * 1024
                        dcol = (q * NT + t) * 1024
                        for h2 in range(2):
                            dec = psB.tile([128, 512], F32, tag="dec")
                            nc.tensor.matmul(
                                dec[:],
                                lhsT=or_sb[:],
                                rhs=b2_sb[0:1, q * 1024 + h2 * 512:q * 1024 + (h2 + 1) * 512],
                                start=True,
                                stop=False,
                                skip_group_check=True,
                            )
                            for mloc in range(2):
                                m = 4 * q + 2 * h2 + mloc
                                for hc in range(2):
                                    fb = 2 * m + hc
                                    lf = (fb - 8 * q) * 256
                                    nc.tensor.matmul(
                                        dec[:, mloc * 256:(mloc + 1) * 256],
                                        lhsT=hT_sb[:, fb * R + t * 128:fb * R + (t + 1) * 128],
                                        rhs=W2q[:, lf:lf + 256],
                                        start=False,
                                        stop=(hc == 1),
                                        skip_group_check=True,
                                    )
                            hcol = dcol + h2 * 512
                            nc.vector.tensor_copy(
                                dcb_sb[:, hcol:hcol + 512], dec[:]
                            )
                            for mloc in range(2):
                                for vh in range(2):
                                    pend_trace.append((
                                        S_ps[:, vh * 256:(vh + 1) * 256],
                                        Tt[:, tb + (2 * h2 + mloc) * 256 + vh * 128:
                                           tb + (2 * h2 + mloc) * 256 + (vh + 1) * 128],
                                        dcb_sb[:, hcol + mloc * 256:hcol + (mloc + 1) * 256],
                                    ))
                        # sev sums for this (q,t): one bf16 exp + DVE m-reduce
                        esb = wT.tile([128, 1024], BF16, tag="esb",
                                      bufs=2 if SILU_VIA_SIGMOID else 3)
                        nc.scalar.activation(
                            esb[:], dcb_sb[:, dcol:dcol + 1024], AF.Exp
                        )
                        pend_sev.append((sev[t][:, q * 4:(q + 1) * 4], esb))
                    # seg sums for this q (rhs dcb slices all ready)
                    for h in range(2):
                        seg_ps = psSg.tile([C, 512], F32, tag="seg")
                        for t in range(NT):
                            dcol = (q * NT + t) * 1024
                            nc.tensor.matmul(
                                seg_ps[:],
                                lhsT=Y_sb[:, t * C:(t + 1) * C],
                                rhs=dcb_sb[:, dcol + h * 512:dcol + (h + 1) * 512],
                                start=(t == 0),
                                stop=(t == NT - 1),
                                skip_group_check=True,
                            )
                        sg_sb = wSg.tile([C, 512], F32, tag="sgev", bufs=1)
                        nc.vector.tensor_copy(sg_sb[:], seg_ps[:])
                        nc.sync.dma_start(
                            seg_part[q][:, h * 512:(h + 1) * 512], sg_sb[:]
                        )
                    if sim_mode:
                        if PER_Q_AR or q == 3:
                            nc.sync.dma_start(seg_red[q][:], seg_part[q][:])
                        if not PER_Q_AR and q == 3:
                            for q2 in range(3):
                                nc.sync.dma_start(seg_red[q2][:], seg_part[q2][:])
                    elif PER_Q_AR:
                        nc.gpsimd.collective_compute(
                            "AllReduce",
                            ALU.add,
                            replica_groups=[list(range(NCORES))],
                            ins=[seg_part[q].opt()],
                            outs=[seg_red[q].opt()],
                        )
                    elif q == 3:
                        nc.gpsimd.collective_compute(
                            "AllReduce",
                            ALU.add,
                            replica_groups=[list(range(NCORES))],
                            ins=[seg_part_f.opt()],
                            outs=[seg_red_f.opt()],
                        )
                    # entropy chunk for this q, pipelined behind its AllReduce
                    sgr = wSg.tile([C, 1024], F32, tag="sgr", bufs=1)
                    nc.sync.dma_start(sgr[:], seg_red[q][:])
                    mns = wSg.tile([C, 1024], BF16, tag="mns", bufs=1)
                    nc.vector.tensor_scalar(
                        mns[:], sgr[:], ic_sb[0:C, 0:1], None, ALU.mult
                    )
                    eE = wSg.tile([C, 1024], BF16, tag="eE", bufs=1)
                    pEs = wSg.tile([C, 1024], BF16, tag="pEs", bufs=1)
                    for mi in range(4):
                        sl = slice(mi * 256, (mi + 1) * 256)
                        cc = q * 4 + mi
                        nc.scalar.activation(
                            eE[:, sl], mns[:, sl], AF.Exp,
                            accum_out=SmE[0:C, cc:cc + 1],
                        )
                        if USE_TTR:
                            nc.vector.tensor_tensor_reduce(
                                pEs[:, sl], eE[:, sl], mns[:, sl], 1.0, 0.0,
                                ALU.mult, ALU.add,
                                accum_out=T1E[0:C, cc:cc + 1],
                            )
                        else:
                            nc.vector.tensor_tensor(
                                pEs[:, sl], eE[:, sl], mns[:, sl], ALU.mult
                            )
                            nc.vector.tensor_reduce(
                                T1E[0:C, cc:cc + 1], pEs[:, sl], AX.X, ALU.add
                            )
                    for h in range(2):
                        dmt = psSg.tile([C, 512], F32, tag="seg")
                        nc.tensor.matmul(
                            dmt[0:1, :],
                            lhsT=of_sb[0:C, 0:1],
                            rhs=sgr[:, h * 512:(h + 1) * 512],
                            start=True,
                            stop=True,
                        )
                        edm = wSg.tile([1, 512], BF16, tag="edm", bufs=1)
                        pdm = wSg.tile([1, 512], BF16, tag="pdm", bufs=1)
                        for mi2 in range(2):
                            mi = 2 * h + mi2
                            cc = q * 4 + mi
                            sl2 = slice(mi2 * 256, (mi2 + 1) * 256)
                            nc.scalar.activation(
                                edm[0:1, sl2], dmt[0:1, sl2], AF.Exp,
                                scale=1.0 / N,
                                accum_out=Sdm[0:1, cc:cc + 1],
                            )
                            if USE_TTR:
                                nc.vector.tensor_tensor_reduce(
                                    pdm[0:1, sl2], dmt[0:1, sl2], edm[0:1, sl2],
                                    1.0 / N, 0.0, ALU.mult, ALU.add,
                                    accum_out=T1d[0:1, cc:cc + 1],
                                )
                            else:
                                nc.vector.scalar_tensor_tensor(
                                    pdm[0:1, sl2], dmt[0:1, sl2], 1.0 / N,
                                    edm[0:1, sl2], ALU.mult, ALU.mult,
                                )
                                nc.vector.tensor_reduce(
                                    T1d[0:1, cc:cc + 1], pdm[0:1, sl2],
                                    AX.X, ALU.add,
                                )
                flush_pending()
                nc.vector.tensor_copy(S_sb[:], S_ps[:])
                # mapLoss pieces that don't need the collective: ln(sev) sums
                # and the trace of S
                for t in range(NT):
                    lns = wSg.tile([128, 16], F32, tag="lns", bufs=2)
                    nc.scalar.activation(lns[:], sev[t][:], AF.Ln)
                    nc.vector.tensor_reduce(
                        mls_sb[:, t:t + 1], lns[:], AX.X, ALU.add
                    )
                trc = cst.tile([128, 1], F32, name="trc")
                dsc = wSg.tile([128, 512], F32, tag="dsc", bufs=1)
                if USE_TTR:
                    nc.vector.tensor_tensor_reduce(
                        dsc[:], S_sb[:], dg_sb[:], 1.0, 0.0,
                        ALU.mult, ALU.add, accum_out=trc[:],
                    )
                else:
                    nc.vector.tensor_tensor(
                        dsc[:], S_sb[:], dg_sb[:], ALU.mult
                    )
                    nc.vector.tensor_reduce(trc[:], dsc[:], AX.X, ALU.add)

            # ---------------- final combine + entropy ----------------
            with (
                tc.tile_pool(name="wE", bufs=2) as wE,
                tc.tile_pool(name="psE", bufs=2, space="PSUM") as psE,
            ):
                lnS = wE.tile([128, NT], F32, tag="lnS")
                nc.scalar.activation(lnS[:], Scols[:], AF.Ln)
                bc = wE.tile([128, NT], F32, tag="bc")
                nc.vector.scalar_tensor_tensor(
                    bc[:], Mcols[:], ITEMP, lnS[:], ALU.mult, ALU.add
                )
                rr = wE.tile([128, 4], F32, tag="rr")
                nc.vector.tensor_reduce(rr[:, 0:1], bc[:], AX.X, ALU.add)
                nc.vector.tensor_reduce(rr[:, 1:2], pb_sb[:], AX.X, ALU.add)
                nc.vector.tensor_reduce(rr[:, 2:3], mls_sb[:], AX.X, ALU.add)
                cmb = wE.tile([128, 2], F32, tag="cmb")
                nc.vector.scalar_tensor_tensor(
                    cmb[:, 0:1], rr[:, 1:2], -ITEMP, rr[:, 0:1], ALU.mult, ALU.add
                )
                nc.vector.tensor_tensor(
                    cmb[:, 1:2], rr[:, 2:3], trc[:], ALU.subtract
                )
                fin_ps = psE.tile([1, 2], F32, tag="fin", bufs=1)
                nc.tensor.matmul(fin_ps[:], lhsT=of_sb[:], rhs=cmb[:], start=True, stop=True)
                nc.vector.tensor_copy(out_sb[:, 0:2], fin_ps[:])

                # entropy over all-reduced segment sums
                # H = ln(S) - T1/S, masked; intra = colsum; net = sum(LAM*intra - Hdm)
                siE = wE.tile([128, 16], F32, tag="siE")
                nc.vector.reciprocal(siE[0:C, :], SmE[0:C, :])
                lsE = wE.tile([128, 16], F32, tag="lsE")
                nc.scalar.activation(lsE[0:C, :], SmE[0:C, :], AF.Ln)
                tE = wE.tile([128, 16], F32, tag="tE")
                nc.vector.tensor_tensor(tE[0:C, :], T1E[0:C, :], siE[0:C, :], ALU.mult)
                hE = wE.tile([128, 16], F32, tag="hE")
                nc.vector.tensor_tensor(hE[0:C, :], lsE[0:C, :], tE[0:C, :], ALU.subtract)
                nc.vector.tensor_scalar(
                    hE[0:C, :], hE[0:C, :], cp_sb[0:C, 0:1], None, ALU.mult
                )
                intra_ps = psE.tile([1, 16], F32, tag="intra", bufs=1)
                nc.tensor.matmul(
                    intra_ps[:], lhsT=of_sb[0:C, 0:1], rhs=hE[0:C, :], start=True, stop=True
                )
                sid = wE.tile([1, 16], F32, tag="sid")
                nc.vector.reciprocal(sid[:], Sdm[:])
                lsd = wE.tile([1, 16], F32, tag="lsd")
                nc.scalar.activation(lsd[:], Sdm[:], AF.Ln)
                tdm = wE.tile([1, 16], F32, tag="tdm")
                nc.vector.tensor_tensor(tdm[:], T1d[:], sid[:], ALU.mult)
                hdm = wE.tile([1, 16], F32, tag="hdm")
                nc.vector.tensor_tensor(hdm[:], lsd[:], tdm[:], ALU.subtract)
                ntm = wE.tile([1, 16], F32, tag="ntm")
                nc.vector.scalar_tensor_tensor(
                    ntm[:], intra_ps[:], LAM, hdm[:], ALU.mult, ALU.subtract
                )
                nc.vector.tensor_reduce(out_sb[0:1, 2:3], ntm[:], AX.X, ALU.add)
                nc.sync.dma_start(outv[:], out_sb[:])

    nc.finalize()
    _CACHE[key] = nc
    return nc


def _host_prep(b, z, y, W1, b1, W2, b2, perm_idx, flip_mask_mapper, flip_mask_outer):
    """Build the 8 per-core input maps (layout/cast/index work only)."""
    b = np.asarray(b, np.float32)
    z = np.asarray(z, np.float32)
    y = np.asarray(y).astype(np.int64)
    W1 = np.asarray(W1, np.float32)
    b1 = np.asarray(b1, np.float32)
    W2 = np.asarray(W2, np.float32)
    b2 = np.asarray(b2, np.float32)
    perm_idx = np.asarray(perm_idx).astype(np.int64)
    fm = np.asarray(flip_mask_mapper).astype(bool)
    fo = np.asarray(flip_mask_outer).astype(bool)

    # first-same-class target index per row
    first = np.full(C, -1, np.int64)
    second = np.full(C, -1, np.int64)
    for j in range(N):
        c = y[j]
        if first[c] < 0:
            first[c] = j
        elif second[c] < 0:
            second[c] = j
    t_idx = np.empty(N, np.int64)
    for i in range(N):
        f = first[y[i]]
        if f != i:
            t_idx[i] = f
        elif second[y[i]] >= 0:
            t_idx[i] = second[y[i]]
        else:
            t_idx[i] = 1 if i == 0 else 0

    zp = z[:, perm_idx]
    zflip = np.where(fm, -zp, zp)
    raw = np.where(fo, -zp, zp)
    binary = (raw > 0).reshape(N, M, 8)
    target = (binary * (2 ** np.arange(8, dtype=np.int64))).sum(-1)  # [N, M]
    Tsel = np.zeros((N, 4096), bf16)
    Tsel[np.arange(N)[:, None], np.arange(M) * 256 + target] = 1
    # pre-tiled per-core layout: TselR[p, ((q*NT+t)*1024 + c)] = core-slice[t*128+p, q*1024+c]


    bT = np.ascontiguousarray(b.T).astype(bf16)          # [128, N]
    btT = np.ascontiguousarray(b[t_idx].T).astype(bf16)  # [128, N]
    zfT = np.ascontiguousarray(zflip.T).astype(bf16)
    Y = np.zeros((N, C), bf16)
    Y[np.arange(N), y] = 1

    W1bd = np.zeros((128, 4096), np.float32)
    for m in range(M):
        W1bd[8 * m:8 * m + 8, 256 * m:256 * m + 256] = W1[m]
    W1bd = W1bd.astype(bf16)
    W2s = np.zeros((128, 32 * 256), np.float32)
    for m in range(M):
        for hc in range(2):
            W2s[:, (2 * m + hc) * 256:(2 * m + hc + 1) * 256] = W2[m, hc * 128:(hc + 1) * 128, :]
    W2s = W2s.astype(bf16)
    b1c = np.ascontiguousarray(b1.reshape(4096).reshape(32, 128).T).astype(np.float32)
    b2r = b2.reshape(1, 4096).astype(bf16)
    bigI = (NEG_BIG * np.eye(128, dtype=np.float32)).astype(bf16)
    diagm = np.zeros((128, 512), np.float32)
    for p in range(128):
        diagm[p, p] = 1.0            # v-half 0: diag of S[p, :] at o = p
        diagm[p, 256 + 128 + p] = 1.0  # v-half 1: diag of S[128+p, :] at o = 128+p
    diagm = diagm.astype(bf16)
    counts = np.bincount(y, minlength=C).astype(np.float32)
    invc = np.zeros((128, 1), np.float32)
    invc[:C, 0] = 1.0 / np.clip(counts, 1.0, None)
    cpos = np.zeros((128, 1), np.float32)
    cpos[:C, 0] = (counts > 0).astype(np.float32)
    ones_f = np.ones((128, 1), np.float32)
    ones_r = np.ones((1, 128), bf16)

    in_maps = []
    for core in range(NCORES):
        sl = slice(core * R, (core + 1) * R)
        in_maps.append(
            dict(
                bTrot=np.ascontiguousarray(np.roll(bT, -core * R, axis=1)),
                btT=np.ascontiguousarray(btT[:, sl]),
                zfT=np.ascontiguousarray(zfT[:, sl]),
                Yb=np.ascontiguousarray(Y[sl]),
                Tsel=np.ascontiguousarray(
                    Tsel[sl].reshape(NT, 128, 4, 1024)
                    .transpose(1, 2, 0, 3).reshape(128, 32 * 1024)
                ),
                W1bd=W1bd,
                W2s=W2s,
                b1c=b1c,
                b2r=b2r,
                bigI=bigI,
                diagm=diagm,
                invc=invc,
                cpos=cpos,
                ones_f=ones_f,
                ones_r=ones_r,
            )
        )
    return in_maps


def kernel(**inputs) -> np.ndarray:
    nc = _build_program()
    in_maps = _host_prep(**inputs)
    _CACHE["last_in_maps"] = in_maps
    res = run_bass_kernel_spmd(nc, in_maps, list(range(NCORES)))
    _CACHE["last_results"] = res
    outs = [r["outv"] for r in res.results]
    base_sum = sum(float(o[0, 0]) for o in outs)
    mls_sum = sum(float(o[0, 1]) for o in outs)
    net = float(outs[0][0, 2])
    loss = base_sum / N + mls_sum / N + net
    return np.float32(loss)


def measure_hw_ns(n_iter=30):
    """Device-resident repeated execution timing (min wall per call).

    Test-harness helper only; includes PJRT dispatch overhead, so it is an
    upper bound on true on-device exec time.
    """
    import time
    import jax
    from jax.sharding import Mesh, PartitionSpec, NamedSharding
    from jax.experimental.shard_map import shard_map
    from concourse import bass2jax as b2j
    import concourse.mybir as mybir_

    nc = _build_program()
    in_maps = _CACHE["last_in_maps"]
    b2j.install_neuronx_cc_hook()

    partition_name = nc.partition_id_tensor.name if nc.partition_id_tensor else None
    in_names, out_names, out_avals, zero_outs = [], [], [], []
    for alloc in nc.m.functions[0].allocations:
        if not isinstance(alloc, mybir_.MemoryLocationSet):
            continue
        name = alloc.memorylocations[0].name
        if alloc.kind == "ExternalInput":
            if name != partition_name:
                in_names.append(name)
        elif alloc.kind == "ExternalOutput":
            shape = tuple(alloc.tensor_shape)
            np_dt = mybir_.dt.np(alloc.dtype)
            out_names.append(name)
            out_avals.append(jax.core.ShapedArray(shape, np_dt))
            zero_outs.append(np.zeros(shape, np_dt))
    n_params = len(in_names)
    n_outs = len(out_names)
    all_in_names = list(in_names) + list(out_names)
    if partition_name is not None:
        all_in_names.append(partition_name)

    def _body(*args):
        operands = list(args)
        if partition_name is not None:
            operands.append(b2j.partition_id_tensor())
        outs = b2j._bass_exec_p.bind(
            *operands,
            out_avals=tuple(out_avals),
            in_names=tuple(all_in_names),
            out_names=tuple(out_names),
            lowering_input_output_aliases=(),
            sim_require_finite=True,
            sim_require_nnan=True,
            nc=nc,
        )
        return tuple(outs)

    devices = jax.devices()[:NCORES]
    mesh = Mesh(np.asarray(devices), ("core",))
    in_specs = (PartitionSpec("core"),) * (n_params + n_outs)
    out_specs = (PartitionSpec("core"),) * n_outs
    fn = jax.jit(
        shard_map(_body, mesh=mesh, in_specs=in_specs, out_specs=out_specs,
                  check_rep=False),
        keep_unused=True,
    )
    per_core = [[np.asarray(m[name]) for name in in_names] for m in in_maps]
    concat_in = [
        np.concatenate([per_core[c][i] for c in range(NCORES)], axis=0)
        for i in range(n_params)
    ]
    concat_zeros = [
        np.zeros((NCORES * z.shape[0], *z.shape[1:]), z.dtype) for z in zero_outs
    ]
    sh = NamedSharding(mesh, PartitionSpec("core"))
    dev_in = [jax.device_put(a, sh) for a in concat_in]
    dev_zero = [jax.device_put(a, sh) for a in concat_zeros]
    # warmup (compile + first runs)
    for _ in range(3):
        r = fn(*dev_in, *dev_zero)
        jax.block_until_ready(r)
    times = []
    for _ in range(n_iter):
        t0 = time.perf_counter()
        r = fn(*dev_in, *dev_zero)
        jax.block_until_ready(r)
        times.append(time.perf_counter() - t0)
    times.sort()
    return dict(
        min_ns=int(times[0] * 1e9),
        p50_ns=int(times[len(times) // 2] * 1e9),
        mean_ns=int(sum(times) / len(times) * 1e9),
    )
